# revision 1
# baseline (speedup 1.0000x reference)
"""Trainium2 Bass kernel for nn_AtomfeatsToRotmat: 2-layer MLP -> batched 3x3
special procrustes (nearest SO(3) rotation), via the Davenport q-method:
R = R(q) where q = top eigenvector of the 4x4 Davenport matrix K(M).
lambda_max is computed in closed form (Cardano on the eigenvalues of M^T M:
lam = s1 + s2 + sign(det M)*s3) + 2 Newton polish steps on K's characteristic
quartic; q is extracted from 2 adjugate columns of (K - lam I) with norm-based
selection. Fully branchless, vectorized over matrices in plane-major layout.

Sharding: data-parallel over nodes across 8 cores. X is transposed on the host
so the device streams [d, n] tiles directly (no on-chip transposes).
"""
import math
import numpy as np

import concourse.bacc as bacc
import concourse.mybir as mybir
from concourse import tile
from concourse.bass_utils import run_bass_kernel_spmd

F32 = mybir.dt.float32
F32R = mybir.dt.float32r
AF = mybir.ActivationFunctionType
ALU = mybir.AluOpType

N_NODES = 262144
D = 256
NCORES = 8


def build_nc(nodes_per_core: int, num_devices: int = NCORES, gelu_fn=None, mlp_only=False):
    """Build the Bass program for one core processing `nodes_per_core` nodes."""
    NPC = nodes_per_core
    assert NPC % 4096 == 0 or NPC in (2048, 1024)
    if gelu_fn is None:
        gelu_fn = AF.Gelu
    CHUNK = 512
    NBATCH = 4 if NPC >= 16384 else (2 if NPC >= 8192 else 1)  # super-batches
    NPB = NPC // NBATCH                # nodes per batch
    NCH_B = NPB // CHUNK               # MLP chunks per batch
    GRP_CH = min(8, NCH_B)             # chunks per plane-scatter group
    NGRP_B = NCH_B // GRP_CH
    GRP_NODES = GRP_CH * CHUNK         # nodes per group
    F = NPB // 128                     # free size of one per-batch plane
    PPG = GRP_NODES // F               # partitions covered by one group

    nc = bacc.Bacc("TRN2", target_bir_lowering=False, debug=False,
                   num_devices=num_devices)
    F16 = mybir.dt.float16
    xh = nc.declare_dram_parameter("xh", [256, NPC], F16, isOutput=False)
    xl = nc.declare_dram_parameter("xl", [256, NPC], F16, isOutput=False)
    w1h = nc.declare_dram_parameter("w1h", [256, 256], F16, isOutput=False)
    w1l = nc.declare_dram_parameter("w1l", [256, 256], F16, isOutput=False)
    b1 = nc.declare_dram_parameter("b1", [256, 1], F32, isOutput=False)
    w2t = nc.declare_dram_parameter("w2t", [256, 16], F32, isOutput=False)
    b2 = nc.declare_dram_parameter("b2", [16, 1], F32, isOutput=False)
    r_out = nc.declare_dram_parameter("r_out", [9, NPC], F32, isOutput=True)

    with tile.TileContext(nc) as tc:
        with (
            tc.tile_pool(name="const", bufs=1) as cpool,
            tc.tile_pool(name="x", bufs=3) as xpool,
            tc.tile_pool(name="h", bufs=6) as hpool,
            tc.tile_pool(name="rawg", bufs=2) as rawpool,
            tc.tile_pool(name="psh", bufs=4, space="PSUM") as pspool,
            tc.tile_pool(name="psr", bufs=4, space="PSUM") as ps2pool,
            tc.tile_pool(name="planes", bufs=2) as ppool,
            tc.tile_pool(name="chain", bufs=2) as qpool,
        ):
            # ---- constants ----
            w1h_sb = cpool.tile([128, 2, 256], F16, tag="w1h")
            nc.sync.dma_start(w1h_sb[:], w1h.rearrange("(a p) o -> p a o", p=128))
            w1l_sb = cpool.tile([128, 2, 256], F16, tag="w1l")
            nc.sync.dma_start(w1l_sb[:], w1l.rearrange("(a p) o -> p a o", p=128))
            b1_sb = cpool.tile([128, 2], F32, tag="b1")
            nc.sync.dma_start(b1_sb[:], b1.rearrange("(a p) one -> p (a one)", p=128))
            w2t_sb = cpool.tile([128, 2, 16], F32, tag="w2t")
            nc.sync.dma_start(w2t_sb[:], w2t.rearrange("(a p) o -> p a o", p=128))
            b2_sb = cpool.tile([16, 1], F32, tag="b2")
            nc.sync.dma_start(b2_sb[:], b2[:, :])
            shift_sb = cpool.tile([128, 2], F32, tag="shift")
            nc.gpsimd.memset(shift_sb[:, 0:1], math.pi/2)
            nc.gpsimd.memset(shift_sb[:, 1:2], -math.pi/6)

            for bi in range(NBATCH):
                # plane-major raw M entries: P[:, e, :] is the plane of entry e
                P = ppool.tile([128, 9, F], F32, tag="P")

                # ================= Phase A: MLP =================
                for g in range(NGRP_B):
                    raw_g = rawpool.tile([16, GRP_NODES], F32, tag="rawg")
                    for qg in range(GRP_CH // 4):      # 4 chunks per col-tiled L2 bank
                        h_quads = []
                        for cc4 in range(4):
                            cc = qg * 4 + cc4
                            ch = bi * NCH_B + g * GRP_CH + cc
                            xh_sb = xpool.tile([128, 2, CHUNK], F16, tag="xh")
                            nc.sync.dma_start(
                                xh_sb[:],
                                xh[:, ch*CHUNK:(ch+1)*CHUNK].rearrange("(a p) n -> p a n", p=128),
                            )
                            xl_sb = xpool.tile([128, 2, CHUNK], F16, tag="xl")
                            nc.sync.dma_start(
                                xl_sb[:],
                                xl[:, ch*CHUNK:(ch+1)*CHUNK].rearrange("(a p) n -> p a n", p=128),
                            )
                            h_sb = hpool.tile([128, 2, CHUNK], F32, tag="h")
                            for oh in range(2):
                                psum_h = pspool.tile([128, CHUNK], F32, tag="psh")
                                terms = [(w1h_sb, xh_sb), (w1l_sb, xh_sb), (w1h_sb, xl_sb)]
                                nmm = 0
                                for wsb, xsb in terms:
                                    for dh in range(2):
                                        nc.tensor.matmul(
                                            psum_h[:],
                                            wsb[:, dh, oh*128:(oh+1)*128],
                                            xsb[:, dh, :],
                                            start=(nmm == 0), stop=(nmm == 5),
                                        )
                                        nmm += 1
                                nc.scalar.activation(h_sb[:, oh, :], psum_h[:], gelu_fn,
                                                     bias=b1_sb[:, oh:oh+1])
                            h_quads.append(h_sb)
                        # col-tiled L2: 4 chunks -> 4 PE column groups (own banks)
                        psum_quads = []
                        for j in range(4):
                            psum_r = ps2pool.tile([128, CHUNK], F32, tag="psr")
                            for oh in range(2):
                                nc.tensor.matmul(
                                    psum_r[32*j:32*j+9, :],
                                    w2t_sb[:, oh, 0:9],
                                    h_quads[j][:, oh, :],
                                    start=(oh == 0), stop=(oh == 1),
                                    tile_position=(0, 32*j),
                                )
                            psum_quads.append(psum_r)
                        for j in range(4):
                            cc = qg * 4 + j
                            dst = raw_g[0:9, cc*CHUNK:(cc+1)*CHUNK]
                            if j % 2 == 0:
                                nc.scalar.activation(dst, psum_quads[j][32*j:32*j+9, :],
                                                     AF.Identity, bias=b2_sb[0:9, 0:1])
                            else:
                                nc.vector.tensor_scalar(dst, psum_quads[j][32*j:32*j+9, :],
                                                        b2_sb[0:9, 0:1], None, ALU.add)
                    # scatter group rows into plane-major P
                    for e in range(9):
                        nc.sync.dma_start(P[g*PPG:(g+1)*PPG, e, :], raw_g[e:e+1, :])

                if mlp_only:
                    for e in range(9):
                        nc.sync.dma_start(r_out[e, bi*NPB:(bi+1)*NPB], P[:, e, :])
                    continue

                # ================= Phase B: SVD chain =================
                V = nc.vector       # DVE
                S = nc.scalar       # ACT
                G = nc.gpsimd       # POOL

                def t(shape_planes, tag):
                    return qpool.tile([128, shape_planes, F], F32, tag=tag, name=tag)

                m = [P[:, e, :] for e in range(9)]
                P_all = P[:, :, :].rearrange("p e f -> p (e f)")

                # --- squares of m and SS = tr(M^T M) ---
                SQ = t(9, "SQ"); SQ_all = SQ[:, :, :].rearrange("p e f -> p (e f)")
                S.activation(SQ_all, P_all, AF.Square)
                t4 = t(4, "t4"); t4_all = t4[:, :, :].rearrange("p e f -> p (e f)")
                V.tensor_tensor(t4_all, SQ[:, 0:4, :].rearrange("p e f -> p (e f)"),
                                SQ[:, 4:8, :].rearrange("p e f -> p (e f)"), ALU.add)
                t2 = t(2, "t2"); t2_all = t2[:, :, :].rearrange("p e f -> p (e f)")
                V.tensor_tensor(t2_all, t4[:, 0:2, :].rearrange("p e f -> p (e f)"),
                                t4[:, 2:4, :].rearrange("p e f -> p (e f)"), ALU.add)
                SS = t(1, "SS")[:, 0, :]
                V.tensor_tensor(SS, t2[:, 0, :], t2[:, 1, :], ALU.add)
                V.tensor_tensor(SS, SS, SQ[:, 8, :], ALU.add)

                # --- cofactors of M (adjugate^T entries, row-expansion signs) ---
                # cof[i*3+j] = (-1)^(i+j) * minor(i,j)
                C = t(9, "C"); Cp = [C[:, k, :] for k in range(9)]
                TA = t(1, "TA")[:, 0, :]; TB = t(1, "TB")[:, 0, :]
                pairs = [
                    (0, 4, 8, 5, 7), (1, 5, 6, 3, 8), (2, 3, 7, 4, 6),
                    (3, 7, 2, 1, 8), (4, 0, 8, 2, 6), (5, 1, 6, 0, 7),
                    (6, 1, 5, 2, 4), (7, 2, 3, 0, 5), (8, 0, 4, 1, 3),
                ]
                # cof[k] = m[a]*m[b] - m[c]*m[d] per the (k,a,b,c,d) tuples
                TC = t(1, "TC")[:, 0, :]; TD = t(1, "TD")[:, 0, :]
                for k, a, b, c_, d_ in pairs:
                    eng, sA, sB = (G, TC, TD) if k % 2 == 0 else (V, TA, TB)
                    eng.tensor_tensor(sA, m[a], m[b], ALU.mult)
                    eng.tensor_tensor(sB, m[c_], m[d_], ALU.mult)
                    eng.tensor_tensor(Cp[k], sA, sB, ALU.subtract)

                # --- det = m0*cof0 + m1*cof1 + m2*cof2 ---
                det = t(1, "det")[:, 0, :]
                V.tensor_tensor(TA, m[0], Cp[0], ALU.mult)
                V.tensor_tensor(TB, m[1], Cp[1], ALU.mult)
                V.tensor_tensor(det, TA, TB, ALU.add)
                V.tensor_tensor(TA, m[2], Cp[2], ALU.mult)
                V.tensor_tensor(det, det, TA, ALU.add)

                # --- adjss = ||adj M||_F^2 ---
                SQ2 = t(9, "SQ2")
                S.activation(SQ2[:, :, :].rearrange("p e f -> p (e f)"),
                             C[:, :, :].rearrange("p e f -> p (e f)"), AF.Square)
                V.tensor_tensor(t4_all, SQ2[:, 0:4, :].rearrange("p e f -> p (e f)"),
                                SQ2[:, 4:8, :].rearrange("p e f -> p (e f)"), ALU.add)
                V.tensor_tensor(t2_all, t4[:, 0:2, :].rearrange("p e f -> p (e f)"),
                                t4[:, 2:4, :].rearrange("p e f -> p (e f)"), ALU.add)
                adjss = t(1, "adjss")[:, 0, :]
                V.tensor_tensor(adjss, t2[:, 0, :], t2[:, 1, :], ALU.add)
                V.tensor_tensor(adjss, adjss, SQ2[:, 8, :], ALU.add)

                # --- quartic coefficients ---
                SS2 = t(1, "SS2")[:, 0, :]
                S.activation(SS2, SS, AF.Square)
                c0 = t(1, "c0")[:, 0, :]
                V.scalar_tensor_tensor(c0, adjss, -4.0, SS2, ALU.mult, ALU.add)
                c1 = t(1, "c1")[:, 0, :]
                V.tensor_scalar(c1, det, -8.0, None, ALU.mult)
                c2 = t(1, "c2")[:, 0, :]
                V.tensor_scalar(c2, SS, -2.0, None, ALU.mult)
                twoc2 = t(1, "twoc2")[:, 0, :]
                V.tensor_scalar(twoc2, SS, -4.0, None, ALU.mult)

                # --- Cardano: eigenvalues of A = M^T M ---
                # cubic d^3 - I1 d^2 + I2 d - I3 ; I1=SS, I2=adjss, I3=det^2
                I3 = t(1, "I3")[:, 0, :]
                S.activation(I3, det, AF.Square)
                bq_t = t(1, "bq"); bq = bq_t[:, 0, :]
                V.tensor_scalar(bq, SS, 1.0/3.0, None, ALU.mult)
                Qq = t(1, "Qq")[:, 0, :]
                V.scalar_tensor_tensor(Qq, adjss, -3.0, SS2, ALU.mult, ALU.add)  # SS^2-3I2
                V.tensor_scalar(Qq, Qq, 1.0/9.0, 0.0, ALU.mult, ALU.max)         # /9, clamp >=0
                I13 = t(1, "I13")[:, 0, :]
                V.tensor_tensor(I13, SS2, SS, ALU.mult)
                I1I2 = t(1, "I1I2")[:, 0, :]
                V.tensor_tensor(I1I2, SS, adjss, ALU.mult)
                Rq = t(1, "Rq")[:, 0, :]
                V.tensor_scalar(I13, I13, 2.0, None, ALU.mult)
                V.scalar_tensor_tensor(Rq, I1I2, -9.0, I13, ALU.mult, ALU.add)
                V.scalar_tensor_tensor(Rq, I3, 27.0, Rq, ALU.mult, ALU.add)
                # ratio = Rq / (54 * Q^{3/2})
                Qs = t(1, "Qs")[:, 0, :]
                S.activation(Qs, Qq, AF.Sqrt)
                Q32 = t(1, "Q32")[:, 0, :]
                V.tensor_tensor(Q32, Qq, Qs, ALU.mult)
                V.tensor_scalar(Q32, Q32, 54.0, None, ALU.mult)
                rinv = t(1, "rinv")[:, 0, :]
                V.reciprocal(rinv, Q32)
                ratio = t(1, "ratio")[:, 0, :]
                V.tensor_tensor(ratio, Rq, rinv, ALU.mult)
                V.tensor_scalar(ratio, ratio, 1.0, -1.0, ALU.min, ALU.max)
                # acos(u) = 2*atan(sqrt((1-u)/(1+u)))
                numq = t(1, "numq")[:, 0, :]
                V.tensor_scalar(numq, ratio, -1.0, 1.0, ALU.mult, ALU.add)
                denq = t(1, "denq")[:, 0, :]
                V.tensor_scalar(denq, ratio, 1.0, 1.0 + 1e-30, ALU.mult, ALU.add)
                V.reciprocal(rinv, denq)
                V.tensor_tensor(numq, numq, rinv, ALU.mult)
                sarg = t(1, "sarg")[:, 0, :]
                S.activation(sarg, numq, AF.Sqrt)
                V.tensor_scalar(sarg, sarg, 10.0, None, ALU.min)
                phi = t(1, "phi")[:, 0, :]
                S.activation(phi, sarg, AF.Arctan)
                V.tensor_scalar(phi, phi, 2.0/3.0, None, ALU.mult)
                # d_k = bq + 2*Qs*cos(phi + shift)
                CB = t(3, "CB")
                S.activation(CB[:, 0, :], phi, AF.Sin, bias=shift_sb[:, 0:1], scale=-1.0)
                S.activation(CB[:, 1, :], phi, AF.Sin, bias=shift_sb[:, 1:2], scale=1.0)
                S.activation(CB[:, 2, :], phi, AF.Sin, bias=shift_sb[:, 1:2], scale=-1.0)
                twoQs_t = t(1, "twoQs"); twoQs = twoQs_t[:, 0, :]
                V.tensor_scalar(twoQs, Qs, 2.0, None, ALU.mult)
                DB = t(3, "DB"); DB_all = DB[:, :, :].rearrange("p e f -> p (e f)")
                tQb = twoQs_t[:, 0:1, :].broadcast_to([128, 3, F])
                bqb = bq_t[:, 0:1, :].broadcast_to([128, 3, F])
                V.tensor_tensor(DB[:, :, :], CB[:, :, :], tQb, ALU.mult)
                V.tensor_tensor(DB[:, :, :], DB[:, :, :], bqb, ALU.add)
                V.tensor_scalar(DB_all, DB_all, 0.0, None, ALU.max)
                SGB = t(3, "SGB")
                S.activation(SGB[:, :, :].rearrange("p e f -> p (e f)"), DB_all, AF.Sqrt)
                sgn = t(1, "sgn")[:, 0, :]
                S.activation(sgn, det, AF.Sign)
                lam = t(1, "lam")[:, 0, :]
                V.tensor_tensor(lam, SGB[:, 0, :], SGB[:, 1, :], ALU.add)
                V.tensor_tensor(TA, SGB[:, 2, :], sgn, ALU.mult)
                V.tensor_tensor(lam, lam, TA, ALU.add)

                # --- Newton polish x2 on quartic ---
                lam2 = t(1, "lam2")[:, 0, :]
                fv = t(1, "fv")[:, 0, :]
                fp = t(1, "fp")[:, 0, :]
                bnd = t(1, "bnd")[:, 0, :]
                V.tensor_scalar(bnd, lam, 0.01, 1e-7, ALU.mult, ALU.add)
                nbnd = t(1, "nbnd")[:, 0, :]
                V.tensor_scalar(nbnd, bnd, -1.0, None, ALU.mult)
                for _ in range(3):
                    S.activation(lam2, lam, AF.Square)
                    V.tensor_tensor(fv, lam2, c2, ALU.add)
                    V.tensor_tensor(fv, fv, lam, ALU.mult)
                    V.tensor_tensor(fv, fv, c1, ALU.add)
                    V.tensor_tensor(fv, fv, lam, ALU.mult)
                    V.tensor_tensor(fv, fv, c0, ALU.add)
                    V.scalar_tensor_tensor(fp, lam2, 4.0, twoc2, ALU.mult, ALU.add)
                    V.tensor_tensor(fp, fp, lam, ALU.mult)
                    V.tensor_tensor(fp, fp, c1, ALU.add)
                    V.tensor_scalar(fp, fp, 1e-20, None, ALU.max)
                    V.reciprocal(rinv, fp)
                    V.tensor_tensor(fv, fv, rinv, ALU.mult)   # delta
                    V.tensor_tensor(fv, fv, bnd, ALU.min)
                    V.tensor_tensor(fv, fv, nbnd, ALU.max)
                    V.tensor_tensor(lam, lam, fv, ALU.subtract)

                # --- N = K - lam I (10 unique entries) ---
                # order: [N00 N01 N02 N03 N11 N12 N13 N22 N23 N33]
                NB = t(10, "NB"); n_ = [NB[:, k, :] for k in range(10)]
                G.tensor_tensor(n_[0], m[0], m[4], ALU.add)        # K00 = m0+m4+m8
                G.tensor_tensor(n_[0], n_[0], m[8], ALU.add)
                G.tensor_tensor(n_[1], m[7], m[5], ALU.subtract)   # K01 = m21-m12
                G.tensor_tensor(n_[2], m[2], m[6], ALU.subtract)   # K02 = m02-m20
                G.tensor_tensor(n_[3], m[3], m[1], ALU.subtract)   # K03 = m10-m01
                G.tensor_tensor(n_[4], m[0], m[4], ALU.subtract)   # K11 = m0-m4-m8
                G.tensor_tensor(n_[4], n_[4], m[8], ALU.subtract)
                G.tensor_tensor(n_[5], m[1], m[3], ALU.add)        # K12 = m01+m10
                G.tensor_tensor(n_[6], m[2], m[6], ALU.add)        # K13 = m02+m20
                G.tensor_tensor(n_[7], m[4], m[0], ALU.subtract)   # K22 = m4-m0-m8
                G.tensor_tensor(n_[7], n_[7], m[8], ALU.subtract)
                G.tensor_tensor(n_[8], m[5], m[7], ALU.add)        # K23 = m12+m21
                G.tensor_tensor(n_[9], m[8], m[0], ALU.subtract)   # K33 = m8-m0-m4
                G.tensor_tensor(n_[9], n_[9], m[4], ALU.subtract)
                for k in (0, 4, 7, 9):
                    G.tensor_tensor(n_[k], n_[k], lam, ALU.subtract)

                # --- adjugate columns 0 and 1 of N ---
                # q0 = [adj00, adj01, adj02, adj03], q1 = [adj01, adj11, adj12, adj13]
                Q0 = t(4, "Q0"); Q1 = t(4, "Q1")
                n00, n01, n02, n03, n11, n12, n13, n22, n23, n33 = n_

                scratch = {id(V): (TA, TB), id(G): (TC, TD)}

                def sym3det(out, a, b, c, d, e, f_, eng=V):
                    """det [[a,b,c],[b,d,e],[c,e,f]] = a(df-e^2) - b(bf-ce) + c(be-cd)"""
                    sA, sB = scratch[id(eng)]
                    eng.tensor_tensor(sA, d, f_, ALU.mult)
                    eng.tensor_tensor(sB, e, e, ALU.mult)
                    eng.tensor_tensor(sA, sA, sB, ALU.subtract)
                    eng.tensor_tensor(out, a, sA, ALU.mult)
                    eng.tensor_tensor(sA, b, f_, ALU.mult)
                    eng.tensor_tensor(sB, c, e, ALU.mult)
                    eng.tensor_tensor(sA, sA, sB, ALU.subtract)
                    eng.tensor_tensor(sA, b, sA, ALU.mult)
                    eng.tensor_tensor(out, out, sA, ALU.subtract)
                    eng.tensor_tensor(sA, b, e, ALU.mult)
                    eng.tensor_tensor(sB, c, d, ALU.mult)
                    eng.tensor_tensor(sA, sA, sB, ALU.subtract)
                    eng.tensor_tensor(sA, c, sA, ALU.mult)
                    eng.tensor_tensor(out, out, sA, ALU.add)

                def gen3det(out, r0, r1, r2, eng=V, negate=False):
                    """det of rows r0,r1,r2 (each a 3-tuple of planes), generic."""
                    sA, sB = scratch[id(eng)]
                    (a0, a1, a2), (b0, b1, b2_), (c0_, c1_, c2_) = r0, r1, r2
                    eng.tensor_tensor(sA, b1, c2_, ALU.mult)
                    eng.tensor_tensor(sB, b2_, c1_, ALU.mult)
                    eng.tensor_tensor(sA, sA, sB, ALU.subtract)
                    eng.tensor_tensor(out, a0, sA, ALU.mult)
                    eng.tensor_tensor(sA, b0, c2_, ALU.mult)
                    eng.tensor_tensor(sB, b2_, c0_, ALU.mult)
                    eng.tensor_tensor(sA, sA, sB, ALU.subtract)
                    eng.tensor_tensor(sA, a1, sA, ALU.mult)
                    eng.tensor_tensor(out, out, sA, ALU.subtract)
                    eng.tensor_tensor(sA, b0, c1_, ALU.mult)
                    eng.tensor_tensor(sB, b1, c0_, ALU.mult)
                    eng.tensor_tensor(sA, sA, sB, ALU.subtract)
                    eng.tensor_tensor(sA, a2, sA, ALU.mult)
                    eng.tensor_tensor(out, out, sA, ALU.add)
                    if negate:
                        eng.tensor_scalar(out, out, -1.0, None, ALU.mult)

                # adj00 = det N[{1,2,3},{1,2,3}]
                sym3det(Q0[:, 0, :], n11, n12, n13, n22, n23, n33)
                # adj01 = -det of rows{0,2,3} cols{1,2,3}
                gen3det(Q0[:, 1, :], (n01, n02, n03), (n12, n22, n23), (n13, n23, n33),
                        negate=True)
                # adj02 = det rows{0,1,3} cols{1,2,3}
                gen3det(Q0[:, 2, :], (n01, n02, n03), (n11, n12, n13), (n13, n23, n33))
                # adj03 = -det rows{0,1,2} cols{1,2,3}
                gen3det(Q0[:, 3, :], (n01, n02, n03), (n11, n12, n13), (n12, n22, n23),
                        negate=True)
                # adj11 = det rows{0,2,3} cols{0,2,3}
                sym3det(Q1[:, 1, :], n00, n02, n03, n22, n23, n33)
                # adj12 = -det rows{0,1,3} cols{0,2,3}
                gen3det(Q1[:, 2, :], (n00, n02, n03), (n01, n12, n13), (n03, n23, n33),
                        negate=True, eng=G)
                # adj13 = det rows{0,1,2} cols{0,2,3}
                gen3det(Q1[:, 3, :], (n00, n02, n03), (n01, n12, n13), (n02, n22, n23),
                        eng=G)
                V.tensor_copy(Q1[:, 0, :], Q0[:, 1, :])

                # --- select column with larger norm^2, normalize, q -> R ---
                Q0_all = Q0[:, :, :].rearrange("p e f -> p (e f)")
                Q1_all = Q1[:, :, :].rearrange("p e f -> p (e f)")
                SQ4 = t(4, "SQ4"); SQ4_all = SQ4[:, :, :].rearrange("p e f -> p (e f)")
                n0s = t(1, "n0s")[:, 0, :]; n1s = t(1, "n1s")[:, 0, :]
                S.activation(SQ4_all, Q0_all, AF.Square)
                V.tensor_tensor(t2_all, SQ4[:, 0:2, :].rearrange("p e f -> p (e f)"),
                                SQ4[:, 2:4, :].rearrange("p e f -> p (e f)"), ALU.add)
                V.tensor_tensor(n0s, t2[:, 0, :], t2[:, 1, :], ALU.add)
                S.activation(SQ4_all, Q1_all, AF.Square)
                V.tensor_tensor(t2_all, SQ4[:, 0:2, :].rearrange("p e f -> p (e f)"),
                                SQ4[:, 2:4, :].rearrange("p e f -> p (e f)"), ALU.add)
                V.tensor_tensor(n1s, t2[:, 0, :], t2[:, 1, :], ALU.add)
                mask_t = t(1, "mask")
                mask = mask_t[:, 0, :]
                V.tensor_tensor(mask, n1s, n0s, ALU.is_gt)
                mb = mask_t[:, 0:1, :].broadcast_to([128, 4, F])
                DIF = t(4, "DIF"); DIF_all = DIF[:, :, :].rearrange("p e f -> p (e f)")
                V.tensor_tensor(DIF_all, Q1_all, Q0_all, ALU.subtract)
                V.tensor_tensor(DIF[:, :, :], DIF[:, :, :], mb, ALU.mult)
                V.tensor_tensor(Q0_all, Q0_all, DIF_all, ALU.add)  # Q0 <- selected q
                V.tensor_tensor(n0s, n0s, n1s, ALU.max)          # selected norm^2
                # inv_norm = rsqrt(n0s) with Newton polish
                V.tensor_scalar(n0s, n0s, 1e-30, None, ALU.add)
                V.reciprocal(TA, n0s)
                r0_t = t(1, "r0_"); r0_ = r0_t[:, 0, :]
                S.activation(r0_, TA, AF.Sqrt)                   # ~ rsqrt(n)
                S.activation(TB, r0_, AF.Square)
                V.tensor_tensor(TB, n0s, TB, ALU.mult)
                V.tensor_scalar(TB, TB, -0.5, 1.5, ALU.mult, ALU.add)
                V.tensor_tensor(r0_, r0_, TB, ALU.mult)
                rb = r0_t[:, 0:1, :].broadcast_to([128, 4, F])
                V.tensor_tensor(Q0[:, :, :], Q0[:, :, :], rb, ALU.mult)  # normalized q

                # doubled products: SQD = 2*q_i^2 via Square(scale=sqrt2);
                # cross: qd = 2*q then qd[i]*q[j]
                SQD = t(4, "SQD")
                S.activation(SQD[:, :, :].rearrange("p e f -> p (e f)"), Q0_all,
                             AF.Square, scale=math.sqrt(2.0))
                QD = t(4, "QD")
                V.tensor_scalar(QD[:, :, :].rearrange("p e f -> p (e f)"), Q0_all, 2.0,
                                None, ALU.mult)
                qw, qx, qy, qz = (Q0[:, k, :] for k in range(4))
                dw, dx, dy, dz = (QD[:, k, :] for k in range(4))
                w2, x2, y2, z2 = (SQD[:, k, :] for k in range(4))  # doubled squares
                XY = t(1, "XY")[:, 0, :]; XZ = t(1, "XZ")[:, 0, :]
                YZ = t(1, "YZ")[:, 0, :]; WX = t(1, "WX")[:, 0, :]
                WY = t(1, "WY")[:, 0, :]; WZ = t(1, "WZ")[:, 0, :]
                V.tensor_tensor(XY, dx, qy, ALU.mult)
                V.tensor_tensor(XZ, dx, qz, ALU.mult)
                G.tensor_tensor(YZ, dy, qz, ALU.mult)
                G.tensor_tensor(WX, dw, qx, ALU.mult)
                G.tensor_tensor(WY, dw, qy, ALU.mult)
                G.tensor_tensor(WZ, dw, qz, ALU.mult)

                RB = t(9, "RB")
                r = [RB[:, k, :] for k in range(9)]
                # diag: r00 = 1-(y2+z2), r11 = 1-(x2+z2), r22 = 1-(x2+y2)
                V.tensor_tensor(TA, y2, z2, ALU.add)
                V.tensor_scalar(r[0], TA, -1.0, 1.0, ALU.mult, ALU.add)
                V.tensor_tensor(TA, x2, z2, ALU.add)
                V.tensor_scalar(r[4], TA, -1.0, 1.0, ALU.mult, ALU.add)
                V.tensor_tensor(TA, x2, y2, ALU.add)
                V.tensor_scalar(r[8], TA, -1.0, 1.0, ALU.mult, ALU.add)
                # off-diag (doubled products already)
                V.tensor_tensor(r[1], XY, WZ, ALU.subtract)   # r01 = 2(xy-wz)
                V.tensor_tensor(r[3], XY, WZ, ALU.add)        # r10
                V.tensor_tensor(r[2], XZ, WY, ALU.add)        # r02
                V.tensor_tensor(r[6], XZ, WY, ALU.subtract)   # r20
                V.tensor_tensor(r[5], YZ, WX, ALU.subtract)   # r12
                V.tensor_tensor(r[7], YZ, WX, ALU.add)        # r21

                for e in range(9):
                    nc.sync.dma_start(r_out[e, bi*NPB:(bi+1)*NPB], RB[:, e, :])
    nc.compile()
    return nc


_CACHE = {}

def _get_nc(npc, ndev):
    key = (npc, ndev)
    if key not in _CACHE:
        _CACHE[key] = build_nc(npc, ndev)
    return _CACHE[key]


def kernel(bb_feats, W1, b1, W2, b2):
    bb_feats = np.asarray(bb_feats, dtype=np.float32)
    W1 = np.asarray(W1, dtype=np.float32)
    b1 = np.asarray(b1, dtype=np.float32)
    W2 = np.asarray(W2, dtype=np.float32)
    b2 = np.asarray(b2, dtype=np.float32)
    N = bb_feats.shape[0]
    npc = N // NCORES

    w1t = np.ascontiguousarray(W1.T)                       # [256, 256]
    w1h = w1t.astype(np.float16)
    w1l = (w1t - w1h.astype(np.float32)).astype(np.float16)
    w2t = np.zeros((256, 16), np.float32); w2t[:, :9] = W2.T
    b2p = np.zeros((16, 1), np.float32); b2p[:9, 0] = b2
    b1p = b1.reshape(256, 1)

    nc = _get_nc(npc, NCORES)
    in_maps = []
    for c in range(NCORES):
        xt_c = np.ascontiguousarray(bb_feats[c*npc:(c+1)*npc].T)  # [256, npc]
        xh_c = xt_c.astype(np.float16)
        xl_c = (xt_c - xh_c.astype(np.float32)).astype(np.float16)
        in_maps.append({"xh": xh_c, "xl": xl_c, "w1h": w1h, "w1l": w1l,
                        "b1": b1p, "w2t": w2t, "b2": b2p})

    res = run_bass_kernel_spmd(nc, in_maps, list(range(NCORES)))
    out = np.empty((N, 3, 3), np.float32)
    for c in range(NCORES):
        r = res.results[c]["r_out"]                        # [9, npc]
        out[c*npc:(c+1)*npc] = r.T.reshape(npc, 3, 3)
    return out


if __name__ == "__main__":
    rng = np.random.default_rng(0)
    bb = rng.standard_normal((N_NODES, D)).astype(np.float32)
    W1 = (rng.standard_normal((D, D)) / np.sqrt(D)).astype(np.float32)
    W2 = (rng.standard_normal((9, D)) / np.sqrt(D)).astype(np.float32)
    out = kernel(bb_feats=bb, W1=W1, b1=np.zeros(D, np.float32),
                 W2=W2, b2=np.zeros(9, np.float32))
    print(out.shape, out[0])

